# revision 6
# baseline (speedup 1.0000x reference)
"""Trainium2 Bass kernel for nn_GCNHealingAgent (3-layer GCN + node MLP + global head).

Strategy (8 NeuronCores, SPMD):
- Nodes dst-sharded across cores (6250/core), padded/permuted per-core into
  K-buckets (K = in-degree+1 padded to a multiple of 4, nodes grouped in
  32-node supergroups) so the segment-sum one-hot matrices form a small
  shared palette of compile-time patterns.
- Per layer: table = (h @ W) * dinv computed per-shard (PE), AllGather of the
  [8*Npad, 64] table through DRAM, per-chunk indirect-DMA gather of 128
  message rows, PE scatter matmul aggT += msg.T @ palette into PSUM, then
  fused dinv-scale + bias + ReLU + residual on DVE/ACT with h kept transposed
  [64, Npad] in SBUF.
- Epilogue: node MLP on device, outputs h / preds per-shard; global mean
  partials reduced on host; tiny 2-layer global head computed on host in f32.

Self-contained: hardcodes all shapes from the problem spec.
"""
import numpy as np

N = 50000
E = 800000
F = 16
D = 64
C = 8
P = 128
SG = 32
NC_NODES = N // C

_cache = {}


# --------------------------------------------------------------------------
# Host-side plan
# --------------------------------------------------------------------------
def _build_plan(edge_index):
    src = edge_index[0].astype(np.int64)
    dst = edge_index[1].astype(np.int64)
    deg = np.bincount(dst, minlength=N).astype(np.float32) + 1.0
    dinv = (1.0 / np.sqrt(deg)).astype(np.float32)

    order = np.argsort(dst, kind="stable")
    src_sorted = src[order]
    starts = np.searchsorted(dst[order], np.arange(N))
    ends = np.searchsorted(dst[order], np.arange(N) + 1)

    slots_count = (ends - starts) + 1            # in-deg + self loop
    K_all = np.maximum(4, ((slots_count + 3) // 4) * 4)

    core_of = np.arange(N) // NC_NODES
    Ks = sorted(set(K_all.tolist()))
    bucket_sg = {}
    for K in Ks:
        mx = 0
        for c in range(C):
            cnt = int(((K_all == K) & (core_of == c)).sum())
            mx = max(mx, (cnt + SG - 1) // SG)
        bucket_sg[K] = mx
    bucket_sg[Ks[0]] += 1                        # guarantee dummies
    Npad = sum(bucket_sg.values()) * SG
    while Npad % P != 0:
        bucket_sg[Ks[0]] += 1
        Npad = sum(bucket_sg.values()) * SG

    # chunk schedule (identical for every core)
    sched = []
    sg_idx = 0
    for K in Ks:
        for _ in range(bucket_sg[K]):
            for i in range(K // 4):
                sched.append((K, i, sg_idx))
            sg_idx += 1
    n_chunks = len(sched)

    # palette
    pat_ids = {}
    pats = []
    for K in Ks:
        for i in range(K // 4):
            m = np.zeros((P, SG), np.float32)
            s = np.arange(P)
            m[s, (i * P + s) // K] = 1.0
            pat_ids[(K, i)] = len(pats)
            pats.append(m)
    pats = np.stack(pats)                        # [npat, 128, 32]

    per_core_pi = []
    for c in range(C):
        nodes_c = np.arange(c * NC_NODES, (c + 1) * NC_NODES)
        pi = np.full(Npad, -1, np.int64)
        pos = 0
        for K in Ks:
            sel = nodes_c[K_all[nodes_c] == K]
            pi[pos:pos + len(sel)] = sel
            pos += bucket_sg[K] * SG
        per_core_pi.append(pi)

    grow = np.full(N, -1, np.int64)
    for c in range(C):
        real = per_core_pi[c] >= 0
        grow[per_core_pi[c][real]] = c * Npad + np.nonzero(real)[0]

    # vectorized slot/index construction
    sched_arr = np.array(sched, np.int64)        # [n_chunks, 3] (K, i, sg)
    s_ar = np.arange(P)
    Kc = sched_arr[:, 0][:, None]
    ic = sched_arr[:, 1][:, None]
    sgc = sched_arr[:, 2][:, None]
    g = ic * P + s_ar[None, :]
    p_local = sgc * SG + g // Kc                 # [n_chunks, P] position
    slot = g % Kc

    idx_arrays, dinv_pi_arr, mask_arr = [], [], []
    for c in range(C):
        pi = per_core_pi[c]
        dinv_pi = np.where(pi >= 0, dinv[np.maximum(pi, 0)], 0.0).astype(np.float32)
        mask = (pi >= 0).astype(np.float32)
        zrow = c * Npad + int(np.nonzero(pi < 0)[0][0])
        v = pi[p_local]                          # [n_chunks, P] node or -1
        vv = np.maximum(v, 0)
        ne = (ends[vv] - starts[vv])
        idx = np.full((n_chunks, P), zrow, np.int64)
        is_edge = (v >= 0) & (slot < ne)
        is_self = (v >= 0) & (slot == ne)
        e_pos = starts[vv] + np.minimum(slot, np.maximum(ne - 1, 0))
        idx[is_edge] = grow[src_sorted[e_pos[is_edge]]]
        idx[is_self] = grow[vv[is_self]]
        idx_arrays.append(np.ascontiguousarray(idx.T).astype(np.int32))  # [P, n_chunks]
        dinv_pi_arr.append(dinv_pi)
        mask_arr.append(mask)

    return dict(Npad=Npad, n_chunks=n_chunks, sched=sched, pats=pats,
                pat_ids=pat_ids, per_core_pi=per_core_pi, idx=idx_arrays,
                dinv_pi=dinv_pi_arr, mask=mask_arr)


# --------------------------------------------------------------------------
# Bass program
# --------------------------------------------------------------------------
def _build_program(plan):
    import concourse.bass as bass
    import concourse.bacc as bacc
    import concourse.mybir as mybir
    import concourse.tile as tile
    from concourse.masks import make_identity

    Npad = plan["Npad"]
    n_chunks = plan["n_chunks"]
    sched = plan["sched"]
    pat_ids = plan["pat_ids"]
    npat = len(plan["pats"])
    nblk = Npad // P
    nwide = (Npad + 511) // 512
    f32 = mybir.dt.float32

    nc = bacc.Bacc("TRN2", target_bir_lowering=False, debug=False, num_devices=C)

    xT_in = nc.dram_tensor("xT", [F, Npad], f32, kind="ExternalInput")
    idx_in = nc.dram_tensor("idx", [P, n_chunks], mybir.dt.int32, kind="ExternalInput")
    pal_in = nc.dram_tensor("pal", [P, npat * SG], f32, kind="ExternalInput")
    dinvnat_in = nc.dram_tensor("dinvnat", [P, nblk], f32, kind="ExternalInput")
    dinvrow_in = nc.dram_tensor("dinvrow", [1, Npad], f32, kind="ExternalInput")
    biasT_in = nc.dram_tensor("biasT", [D, 6], f32, kind="ExternalInput")
    bn2r_in = nc.dram_tensor("bn2r", [1, 13], f32, kind="ExternalInput")
    mask_in = nc.dram_tensor("maskc", [P, nblk], f32, kind="ExternalInput")
    wemb_in = nc.dram_tensor("Wemb", [F, D], f32, kind="ExternalInput")
    wc_in = [nc.dram_tensor(f"Wc{i+1}", [D, D], f32, kind="ExternalInput")
             for i in range(3)]
    wn1_in = nc.dram_tensor("Wn1", [D, D], f32, kind="ExternalInput")
    wn2_in = nc.dram_tensor("Wn2", [D, 13], f32, kind="ExternalInput")

    hout = nc.dram_tensor("h_out", [Npad, D], f32, kind="ExternalOutput")
    pout = nc.dram_tensor("preds_out", [Npad, 16], f32, kind="ExternalOutput")
    gout = nc.dram_tensor("gsum_out", [D, 1], f32, kind="ExternalOutput")

    with tile.TileContext(nc) as tc:
        with (
            tc.tile_pool(name="const", bufs=1) as cp,
            tc.tile_pool(name="big", bufs=1) as bigp,
            tc.tile_pool(name="stage", bufs=1) as stp,
            tc.tile_pool(name="work", bufs=3) as wp,
            tc.tile_pool(name="msgs", bufs=6) as mp,
            tc.tile_pool(name="ps_small", bufs=2, space="PSUM") as psp,
            tc.tile_pool(name="ps_agg", bufs=2, space="PSUM") as pagg,
            tc.tile_pool(name="ps_wide", bufs=2, space="PSUM") as pwide,
            tc.tile_pool(name="ps_g", bufs=1, space="PSUM") as pg,
            tc.tile_pool(name="dram", bufs=2, space="DRAM") as dr,
        ):
            # ---- load constants ----
            idx_t = cp.tile([P, n_chunks], mybir.dt.int32)
            nc.sync.dma_start(out=idx_t[:], in_=idx_in[:])
            pal_t = cp.tile([P, npat * SG], f32)
            nc.sync.dma_start(out=pal_t[:], in_=pal_in[:])
            dinvnat_t = cp.tile([P, nblk], f32)
            nc.sync.dma_start(out=dinvnat_t[:], in_=dinvnat_in[:])
            dinvrow_t = cp.tile([1, Npad], f32)
            nc.sync.dma_start(out=dinvrow_t[:], in_=dinvrow_in[:])
            biasT_t = cp.tile([D, 6], f32)
            nc.sync.dma_start(out=biasT_t[:], in_=biasT_in[:])
            bn2r_t = cp.tile([1, 13], f32)
            nc.sync.dma_start(out=bn2r_t[:], in_=bn2r_in[:])
            mask_t = cp.tile([P, nblk], f32)
            nc.sync.dma_start(out=mask_t[:], in_=mask_in[:])
            xT_t = cp.tile([F, Npad], f32)
            nc.sync.dma_start(out=xT_t[:], in_=xT_in[:])
            wemb_t = cp.tile([F, D], f32)
            nc.sync.dma_start(out=wemb_t[:], in_=wemb_in[:])
            wc_t = []
            for i in range(3):
                t = cp.tile([D, D], f32, tag=f"wc{i}")
                nc.sync.dma_start(out=t[:], in_=wc_in[i][:])
                wc_t.append(t)
            wn1_t = cp.tile([D, D], f32)
            nc.sync.dma_start(out=wn1_t[:], in_=wn1_in[:])
            wn2_t = cp.tile([D, 13], f32)
            nc.sync.dma_start(out=wn2_t[:], in_=wn2_in[:])

            ones1 = cp.tile([1, P], f32)
            nc.vector.memset(ones1[:], 1.0)
            ident = cp.tile([P, P], f32)
            make_identity(nc, ident[:])

            # ---- dinvT broadcast tile [64, Npad] ----
            dinvT = bigp.tile([D, Npad], f32)
            for j in range(nwide):
                w = min(512, Npad - j * 512)
                ps = pwide.tile([D, 512], f32, tag="wide")
                nc.tensor.matmul(out=ps[:, :w], lhsT=ones1[:, :D],
                                 rhs=dinvrow_t[:, j*512:j*512+w],
                                 start=True, stop=True)
                nc.scalar.activation(out=dinvT[:, j*512:j*512+w], in_=ps[:, :w],
                                     func=mybir.ActivationFunctionType.Copy)

            # bn2 tile [128, 13]
            ps = psp.tile([P, D], f32, tag="sm")
            nc.tensor.matmul(out=ps[:, :13], lhsT=ones1[:], rhs=bn2r_t[:],
                             start=True, stop=True)
            bn2_tile = cp.tile([P, 13], f32)
            nc.scalar.activation(out=bn2_tile[:], in_=ps[:, :13],
                                 func=mybir.ActivationFunctionType.Copy)

            # ---- h1T = Wemb.T @ xT + b_emb ----
            hT = bigp.tile([D, Npad], f32)
            for j in range(nwide):
                w = min(512, Npad - j * 512)
                ps = pwide.tile([D, 512], f32, tag="wide")
                nc.tensor.matmul(out=ps[:, :w], lhsT=wemb_t[:],
                                 rhs=xT_t[:, j*512:j*512+w], start=True, stop=True)
                nc.vector.tensor_scalar_add(out=hT[:, j*512:j*512+w],
                                            in0=ps[:, :w],
                                            scalar1=biasT_t[:, 0:1])

            # ---- layers ----
            for l in range(3):
                # table = (h @ Wc) * dinv  (natural layout), staged then DMA'd
                tab_stage = stp.tile([P, nblk, D], f32, tag="tab")
                for b in range(nblk):
                    ps = psp.tile([P, D], f32, tag="sm")
                    nc.tensor.matmul(out=ps[:], lhsT=hT[:, b*P:(b+1)*P],
                                     rhs=wc_t[l][:], start=True, stop=True)
                    nc.vector.tensor_scalar_mul(out=tab_stage[:, b, :], in0=ps[:],
                                                scalar1=dinvnat_t[:, b:b+1])
                shard = dr.tile([Npad, D], f32, tag="shard")
                nc.sync.dma_start(
                    out=shard[:].rearrange("(b p) d -> p b d", p=P),
                    in_=tab_stage[:])
                table = dr.tile([C * Npad, D], f32, tag="table",
                                addr_space="Shared")
                nc.gpsimd.collective_compute(
                    "AllGather", mybir.AluOpType.bypass,
                    replica_groups=[list(range(C))],
                    ins=[shard.opt()], outs=[table.opt()],
                )

                # gather + scatter-matmul; psum block [64, 128] covers 4 sgs
                cur_blk = -1
                ps_ag = None
                for j, (K, i, sg) in enumerate(sched):
                    blk = sg // 4
                    if blk != cur_blk:
                        # flush previous block
                        if ps_ag is not None:
                            _agg_update(nc, mybir, wp, hT, dinvT, biasT_t,
                                        ps_ag, cur_blk, l, Npad)
                        ps_ag = pagg.tile([D, P], f32, tag="agg")
                        cur_blk = blk
                    msg = mp.tile([P, D], f32, tag="msg")
                    nc.gpsimd.indirect_dma_start(
                        out=msg[:], out_offset=None, in_=table[:],
                        in_offset=bass.IndirectOffsetOnAxis(
                            ap=idx_t[:, j:j+1], axis=0),
                    )
                    pid = pat_ids[(K, i)]
                    soff = (sg % 4) * SG
                    nc.tensor.matmul(
                        out=ps_ag[:, soff:soff+SG],
                        lhsT=msg[:],
                        rhs=pal_t[:, pid*SG:(pid+1)*SG],
                        start=(i == 0), stop=(i == K // 4 - 1),
                    )
                _agg_update(nc, mybir, wp, hT, dinvT, biasT_t, ps_ag, cur_blk,
                            l, Npad)

            # ---- node MLP ----
            p1T = bigp.tile([D, Npad], f32)
            for j in range(nwide):
                w = min(512, Npad - j * 512)
                ps = pwide.tile([D, 512], f32, tag="wide")
                nc.tensor.matmul(out=ps[:, :w], lhsT=wn1_t[:],
                                 rhs=hT[:, j*512:j*512+w], start=True, stop=True)
                nc.scalar.activation(out=p1T[:, j*512:j*512+w], in_=ps[:, :w],
                                     func=mybir.ActivationFunctionType.Relu,
                                     bias=biasT_t[:, 4:5])
            pstage = stp.tile([P, nblk, 16], f32, tag="pstage")
            for b in range(nblk):
                ps = psp.tile([P, D], f32, tag="sm")
                nc.tensor.matmul(out=ps[:, :13], lhsT=p1T[:, b*P:(b+1)*P],
                                 rhs=wn2_t[:], start=True, stop=True)
                nc.vector.tensor_add(out=pstage[:, b, :13], in0=ps[:, :13],
                                     in1=bn2_tile[:])
            nc.sync.dma_start(
                out=pout[:].rearrange("(b p) d -> p b d", p=P),
                in_=pstage[:])

            # ---- h transpose to natural + output + gsum ----
            hstage = stp.tile([P, nblk, D], f32, tag="hstage")
            for b in range(nblk):
                ps = psp.tile([P, D], f32, tag="sm")
                nc.tensor.transpose(out=ps[:], in_=hT[:, b*P:(b+1)*P],
                                    identity=ident[:D, :D])
                nc.scalar.activation(out=hstage[:, b, :], in_=ps[:],
                                     func=mybir.ActivationFunctionType.Copy)
            nc.sync.dma_start(
                out=hout[:].rearrange("(b p) d -> p b d", p=P),
                in_=hstage[:])
            ps_g = pg.tile([D, 1], f32, tag="gps")
            for b in range(nblk):
                nc.tensor.matmul(out=ps_g[:], lhsT=hstage[:, b, :],
                                 rhs=mask_t[:, b:b+1],
                                 start=(b == 0), stop=(b == nblk - 1))
            gs = wp.tile([D, 1], f32, tag="gs")
            nc.scalar.activation(out=gs[:], in_=ps_g[:],
                                 func=mybir.ActivationFunctionType.Copy)
            nc.sync.dma_start(out=gout[:], in_=gs[:])

    nc.finalize()
    return nc


def _agg_update(nc, mybir, wp, hT, dinvT, biasT_t, ps_ag, blk, l, Npad):
    """h[:, blk] += relu(aggT * dinvT + bias_l)  (T layout, [64, 128] block)."""
    sl = slice(blk * P, (blk + 1) * P)
    t1 = wp.tile([64, P], mybir.dt.float32, tag="t1")
    nc.vector.tensor_mul(out=t1[:], in0=ps_ag[:], in1=dinvT[:, sl])
    t2 = wp.tile([64, P], mybir.dt.float32, tag="t2")
    nc.scalar.activation(out=t2[:], in_=t1[:],
                         func=mybir.ActivationFunctionType.Relu,
                         bias=biasT_t[:, l+1:l+2])
    nc.vector.tensor_add(out=hT[:, sl], in0=hT[:, sl], in1=t2[:])


# --------------------------------------------------------------------------
# kernel entry
# --------------------------------------------------------------------------
def kernel(x, edge_index, W_emb, b_emb, Wc1, bc1, Wc2, bc2, Wc3, bc3,
           Wn1, bn1, Wn2, bn2, Wg1, bg1, Wg2, bg2):
    from concourse.bass_utils import run_bass_kernel_spmd

    f32 = np.float32
    x = np.asarray(x, f32)
    edge_index = np.asarray(edge_index)
    key = edge_index.tobytes()[:0]  # plans keyed per call; cache on id is risky

    plan = _build_plan(edge_index)
    Npad, n_chunks = plan["Npad"], plan["n_chunks"]
    nblk = Npad // P

    prog_key = (Npad, n_chunks, tuple(plan["pat_ids"].keys()),
                tuple(k for k, _, _ in plan["sched"]))
    if _cache.get("key") == prog_key:
        nc = _cache["nc"]
    else:
        nc = _build_program(plan)
        _cache["key"] = prog_key
        _cache["nc"] = nc

    pal_flat = np.ascontiguousarray(
        plan["pats"].transpose(1, 0, 2).reshape(P, -1)).astype(f32)
    biasT = np.zeros((D, 6), f32)
    for col, b in enumerate([b_emb, bc1, bc2, bc3, bn1]):
        biasT[:, col] = np.asarray(b, f32)
    bn2r = np.asarray(bn2, f32).reshape(1, 13)

    in_maps = []
    for c in range(C):
        pi = plan["per_core_pi"][c]
        xp = np.zeros((Npad, F), f32)
        real = pi >= 0
        xp[real] = x[pi[real]]
        dinv_pi = plan["dinv_pi"][c]
        in_maps.append({
            "xT": np.ascontiguousarray(xp.T),
            "idx": plan["idx"][c],
            "pal": pal_flat,
            "dinvnat": np.ascontiguousarray(
                dinv_pi.reshape(nblk, P).T).astype(f32),
            "dinvrow": dinv_pi.reshape(1, Npad),
            "biasT": biasT,
            "bn2r": bn2r,
            "maskc": np.ascontiguousarray(
                plan["mask"][c].reshape(nblk, P).T).astype(f32),
            "Wemb": np.asarray(W_emb, f32),
            "Wc1": np.asarray(Wc1, f32),
            "Wc2": np.asarray(Wc2, f32),
            "Wc3": np.asarray(Wc3, f32),
            "Wn1": np.asarray(Wn1, f32),
            "Wn2": np.asarray(Wn2, f32),
        })

    res = run_bass_kernel_spmd(nc, in_maps, list(range(C)))

    h_full = np.zeros((N, D), f32)
    preds = np.zeros((N, 13), f32)
    gsum = np.zeros(D, np.float64)
    for c in range(C):
        out = res.results[c]
        pi = plan["per_core_pi"][c]
        real = pi >= 0
        h_full[pi[real]] = out["h_out"][real]
        preds[pi[real]] = out["preds_out"][real][:, :13]
        gsum += out["gsum_out"][:, 0]

    g = (gsum / np.float64(N)).astype(f32)[None, :]
    Wg1 = np.asarray(Wg1, f32); bg1 = np.asarray(bg1, f32)
    Wg2 = np.asarray(Wg2, f32); bg2 = np.asarray(bg2, f32)
    z = np.maximum(g @ Wg1 + bg1, 0.0).astype(f32) @ Wg2 + bg2
    gp = (1.0 / (1.0 + np.exp(-z))).astype(f32)

    return (h_full, preds[:, :10], preds[:, 10:13], gp)


# revision 8
# speedup vs baseline: 1.4120x; 1.4120x over previous
"""Trainium2 Bass kernel for nn_GCNHealingAgent (3-layer GCN + node MLP + global head).

Strategy (8 NeuronCores, SPMD):
- Nodes dst-sharded across cores (6250/core), padded/permuted per-core into
  K-buckets (K = in-degree+1 padded to a multiple of 4, nodes grouped in
  32-node supergroups) so the segment-sum one-hot matrices form a small
  shared palette of compile-time patterns.
- Per layer: table = (h @ W) * dinv computed per-shard (PE), AllGather of the
  [8*Npad, 64] table through DRAM, per-chunk indirect-DMA gather of 128
  message rows, PE scatter matmul aggT += msg.T @ palette into PSUM, then
  fused dinv-scale + bias + ReLU + residual on DVE/ACT with h kept transposed
  [64, Npad] in SBUF.
- Epilogue: node MLP on device, outputs h / preds per-shard; global mean
  partials reduced on host; tiny 2-layer global head computed on host in f32.

Self-contained: hardcodes all shapes from the problem spec.
"""
import numpy as np

N = 50000
E = 800000
F = 16
D = 64
C = 8
P = 128
SG = 32
NC_NODES = N // C

_cache = {}


# --------------------------------------------------------------------------
# Host-side plan
# --------------------------------------------------------------------------
def _build_plan(edge_index):
    src = edge_index[0].astype(np.int64)
    dst = edge_index[1].astype(np.int64)
    deg = np.bincount(dst, minlength=N).astype(np.float32) + 1.0
    dinv = (1.0 / np.sqrt(deg)).astype(np.float32)

    order = np.argsort(dst, kind="stable")
    src_sorted = src[order]
    starts = np.searchsorted(dst[order], np.arange(N))
    ends = np.searchsorted(dst[order], np.arange(N) + 1)

    slots_count = (ends - starts) + 1            # in-deg + self loop
    K_all = np.maximum(4, ((slots_count + 3) // 4) * 4)

    core_of = np.arange(N) // NC_NODES
    Ks = sorted(set(K_all.tolist()))
    bucket_sg = {}
    for K in Ks:
        mx = 0
        for c in range(C):
            cnt = int(((K_all == K) & (core_of == c)).sum())
            mx = max(mx, (cnt + SG - 1) // SG)
        bucket_sg[K] = mx
    bucket_sg[Ks[0]] += 1                        # guarantee dummies
    Npad = sum(bucket_sg.values()) * SG
    while Npad % P != 0:
        bucket_sg[Ks[0]] += 1
        Npad = sum(bucket_sg.values()) * SG

    # chunk schedule (identical for every core)
    sched = []
    sg_idx = 0
    for K in Ks:
        for _ in range(bucket_sg[K]):
            for i in range(K // 4):
                sched.append((K, i, sg_idx))
            sg_idx += 1
    n_chunks = len(sched)

    # palette
    pat_ids = {}
    pats = []
    for K in Ks:
        for i in range(K // 4):
            m = np.zeros((P, SG), np.float32)
            s = np.arange(P)
            m[s, (i * P + s) // K] = 1.0
            pat_ids[(K, i)] = len(pats)
            pats.append(m)
    pats = np.stack(pats)                        # [npat, 128, 32]

    per_core_pi = []
    for c in range(C):
        nodes_c = np.arange(c * NC_NODES, (c + 1) * NC_NODES)
        pi = np.full(Npad, -1, np.int64)
        pos = 0
        for K in Ks:
            sel = nodes_c[K_all[nodes_c] == K]
            pi[pos:pos + len(sel)] = sel
            pos += bucket_sg[K] * SG
        per_core_pi.append(pi)

    grow = np.full(N, -1, np.int64)
    for c in range(C):
        real = per_core_pi[c] >= 0
        grow[per_core_pi[c][real]] = c * Npad + np.nonzero(real)[0]

    # vectorized slot/index construction
    sched_arr = np.array(sched, np.int64)        # [n_chunks, 3] (K, i, sg)
    s_ar = np.arange(P)
    Kc = sched_arr[:, 0][:, None]
    ic = sched_arr[:, 1][:, None]
    sgc = sched_arr[:, 2][:, None]
    g = ic * P + s_ar[None, :]
    p_local = sgc * SG + g // Kc                 # [n_chunks, P] position
    slot = g % Kc

    idx_arrays, dinv_pi_arr, mask_arr = [], [], []
    for c in range(C):
        pi = per_core_pi[c]
        dinv_pi = np.where(pi >= 0, dinv[np.maximum(pi, 0)], 0.0).astype(np.float32)
        mask = (pi >= 0).astype(np.float32)
        zrow = c * Npad + int(np.nonzero(pi < 0)[0][0])
        v = pi[p_local]                          # [n_chunks, P] node or -1
        vv = np.maximum(v, 0)
        ne = (ends[vv] - starts[vv])
        idx = np.full((n_chunks, P), zrow, np.int64)
        is_edge = (v >= 0) & (slot < ne)
        is_self = (v >= 0) & (slot == ne)
        e_pos = starts[vv] + np.minimum(slot, np.maximum(ne - 1, 0))
        idx[is_edge] = grow[src_sorted[e_pos[is_edge]]]
        idx[is_self] = grow[vv[is_self]]
        idx_arrays.append(np.ascontiguousarray(idx.T).astype(np.int32))  # [P, n_chunks]
        dinv_pi_arr.append(dinv_pi)
        mask_arr.append(mask)

    return dict(Npad=Npad, n_chunks=n_chunks, sched=sched, pats=pats,
                pat_ids=pat_ids, per_core_pi=per_core_pi, idx=idx_arrays,
                dinv_pi=dinv_pi_arr, mask=mask_arr)


# --------------------------------------------------------------------------
# Bass program
# --------------------------------------------------------------------------
def _build_program(plan):
    import concourse.bass as bass
    import concourse.bacc as bacc
    import concourse.mybir as mybir
    import concourse.tile as tile
    from concourse.masks import make_identity

    Npad = plan["Npad"]
    n_chunks = plan["n_chunks"]
    sched = plan["sched"]
    pat_ids = plan["pat_ids"]
    npat = len(plan["pats"])
    nblk = Npad // P
    nwide = (Npad + 511) // 512
    f32 = mybir.dt.float32

    nc = bacc.Bacc("TRN2", target_bir_lowering=False, debug=False, num_devices=C)

    xT_in = nc.dram_tensor("xT", [F, Npad], f32, kind="ExternalInput")
    idx_in = nc.dram_tensor("idx", [P, n_chunks], mybir.dt.int32, kind="ExternalInput")
    pal_in = nc.dram_tensor("pal", [P, npat * SG], f32, kind="ExternalInput")
    dinvnat_in = nc.dram_tensor("dinvnat", [P, nblk], f32, kind="ExternalInput")
    dinvrow_in = nc.dram_tensor("dinvrow", [1, Npad], f32, kind="ExternalInput")
    biasT_in = nc.dram_tensor("biasT", [D, 6], f32, kind="ExternalInput")
    bn2r_in = nc.dram_tensor("bn2r", [1, 13], f32, kind="ExternalInput")
    mask_in = nc.dram_tensor("maskc", [P, nblk], f32, kind="ExternalInput")
    wemb_in = nc.dram_tensor("Wemb", [F, D], f32, kind="ExternalInput")
    wc_in = [nc.dram_tensor(f"Wc{i+1}", [D, D], f32, kind="ExternalInput")
             for i in range(3)]
    wn1_in = nc.dram_tensor("Wn1", [D, D], f32, kind="ExternalInput")
    wn2_in = nc.dram_tensor("Wn2", [D, 13], f32, kind="ExternalInput")

    hout = nc.dram_tensor("h_out", [Npad, D], f32, kind="ExternalOutput")
    pout = nc.dram_tensor("preds_out", [Npad, 16], f32, kind="ExternalOutput")
    gout = nc.dram_tensor("gsum_out", [D, 1], f32, kind="ExternalOutput")

    with tile.TileContext(nc) as tc:
        with (
            tc.tile_pool(name="const", bufs=1) as cp,
            tc.tile_pool(name="big", bufs=1) as bigp,
            tc.tile_pool(name="stage", bufs=1) as stp,
            tc.tile_pool(name="work", bufs=3) as wp,
            tc.tile_pool(name="msgs", bufs=12) as mp,
            tc.tile_pool(name="ps_small", bufs=2, space="PSUM") as psp,
            tc.tile_pool(name="ps_agg", bufs=2, space="PSUM") as pagg,
            tc.tile_pool(name="ps_wide", bufs=2, space="PSUM") as pwide,
            tc.tile_pool(name="ps_g", bufs=1, space="PSUM") as pg,
            tc.tile_pool(name="dram", bufs=2, space="DRAM") as dr,
        ):
            # ---- load constants ----
            idx_t = cp.tile([P, n_chunks], mybir.dt.int32)
            nc.sync.dma_start(out=idx_t[:], in_=idx_in[:])
            pal_t = cp.tile([P, npat * SG], f32)
            nc.sync.dma_start(out=pal_t[:], in_=pal_in[:])
            dinvnat_t = cp.tile([P, nblk], f32)
            nc.sync.dma_start(out=dinvnat_t[:], in_=dinvnat_in[:])
            dinvrow_t = cp.tile([1, Npad], f32)
            nc.sync.dma_start(out=dinvrow_t[:], in_=dinvrow_in[:])
            biasT_t = cp.tile([D, 6], f32)
            nc.sync.dma_start(out=biasT_t[:], in_=biasT_in[:])
            bn2r_t = cp.tile([1, 13], f32)
            nc.sync.dma_start(out=bn2r_t[:], in_=bn2r_in[:])
            mask_t = cp.tile([P, nblk], f32)
            nc.sync.dma_start(out=mask_t[:], in_=mask_in[:])
            xT_t = cp.tile([F, Npad], f32)
            nc.sync.dma_start(out=xT_t[:], in_=xT_in[:])
            wemb_t = cp.tile([F, D], f32)
            nc.sync.dma_start(out=wemb_t[:], in_=wemb_in[:])
            wc_t = []
            for i in range(3):
                t = cp.tile([D, D], f32, tag=f"wc{i}")
                nc.sync.dma_start(out=t[:], in_=wc_in[i][:])
                wc_t.append(t)
            wn1_t = cp.tile([D, D], f32)
            nc.sync.dma_start(out=wn1_t[:], in_=wn1_in[:])
            wn2_t = cp.tile([D, 13], f32)
            nc.sync.dma_start(out=wn2_t[:], in_=wn2_in[:])

            ones1 = cp.tile([1, P], f32)
            nc.vector.memset(ones1[:], 1.0)
            ident = cp.tile([P, P], f32)
            make_identity(nc, ident[:])

            # ---- dinvT broadcast tile [64, Npad] ----
            dinvT = bigp.tile([D, Npad], f32)
            for j in range(nwide):
                w = min(512, Npad - j * 512)
                ps = pwide.tile([D, 512], f32, tag="wide")
                nc.tensor.matmul(out=ps[:, :w], lhsT=ones1[:, :D],
                                 rhs=dinvrow_t[:, j*512:j*512+w],
                                 start=True, stop=True)
                nc.scalar.activation(out=dinvT[:, j*512:j*512+w], in_=ps[:, :w],
                                     func=mybir.ActivationFunctionType.Copy)

            # bn2 tile [128, 13]
            ps = psp.tile([P, D], f32, tag="sm")
            nc.tensor.matmul(out=ps[:, :13], lhsT=ones1[:], rhs=bn2r_t[:],
                             start=True, stop=True)
            bn2_tile = cp.tile([P, 13], f32)
            nc.scalar.activation(out=bn2_tile[:], in_=ps[:, :13],
                                 func=mybir.ActivationFunctionType.Copy)

            # ---- h1T = Wemb.T @ xT + b_emb ----
            hT = bigp.tile([D, Npad], f32)
            for j in range(nwide):
                w = min(512, Npad - j * 512)
                ps = pwide.tile([D, 512], f32, tag="wide")
                nc.tensor.matmul(out=ps[:, :w], lhsT=wemb_t[:],
                                 rhs=xT_t[:, j*512:j*512+w], start=True, stop=True)
                nc.vector.tensor_scalar_add(out=hT[:, j*512:j*512+w],
                                            in0=ps[:, :w],
                                            scalar1=biasT_t[:, 0:1])

            # ---- layers ----
            for l in range(3):
                # table = (h @ Wc) * dinv  (natural layout), staged then DMA'd
                tab_stage = stp.tile([P, nblk, D], f32, tag="tab")
                for b in range(nblk):
                    ps = psp.tile([P, D], f32, tag="sm")
                    nc.tensor.matmul(out=ps[:], lhsT=hT[:, b*P:(b+1)*P],
                                     rhs=wc_t[l][:], start=True, stop=True)
                    nc.vector.tensor_scalar_mul(out=tab_stage[:, b, :], in0=ps[:],
                                                scalar1=dinvnat_t[:, b:b+1])
                shard = dr.tile([Npad, D], f32, tag="shard")
                nc.sync.dma_start(
                    out=shard[:].rearrange("(b p) d -> p b d", p=P),
                    in_=tab_stage[:])
                table = dr.tile([C * Npad, D], f32, tag="table")
                nc.gpsimd.collective_compute(
                    "AllGather", mybir.AluOpType.bypass,
                    replica_groups=[list(range(C))],
                    ins=[shard.opt()], outs=[table.opt()],
                )

                # gather + scatter-matmul; psum block [64, 128] covers 4 sgs
                cur_blk = -1
                ps_ag = None
                for j, (K, i, sg) in enumerate(sched):
                    blk = sg // 4
                    if blk != cur_blk:
                        # flush previous block
                        if ps_ag is not None:
                            _agg_update(nc, mybir, wp, hT, dinvT, biasT_t,
                                        ps_ag, cur_blk, l, Npad)
                        ps_ag = pagg.tile([D, P], f32, tag="agg")
                        cur_blk = blk
                    msg = mp.tile([P, D], f32, tag="msg")
                    nc.gpsimd.indirect_dma_start(
                        out=msg[:], out_offset=None, in_=table[:],
                        in_offset=bass.IndirectOffsetOnAxis(
                            ap=idx_t[:, j:j+1], axis=0),
                    )
                    pid = pat_ids[(K, i)]
                    soff = (sg % 4) * SG
                    nc.tensor.matmul(
                        out=ps_ag[:, soff:soff+SG],
                        lhsT=msg[:],
                        rhs=pal_t[:, pid*SG:(pid+1)*SG],
                        start=(i == 0), stop=(i == K // 4 - 1),
                    )
                _agg_update(nc, mybir, wp, hT, dinvT, biasT_t, ps_ag, cur_blk,
                            l, Npad)

            # ---- node MLP ----
            p1T = bigp.tile([D, Npad], f32)
            for j in range(nwide):
                w = min(512, Npad - j * 512)
                ps = pwide.tile([D, 512], f32, tag="wide")
                nc.tensor.matmul(out=ps[:, :w], lhsT=wn1_t[:],
                                 rhs=hT[:, j*512:j*512+w], start=True, stop=True)
                nc.scalar.activation(out=p1T[:, j*512:j*512+w], in_=ps[:, :w],
                                     func=mybir.ActivationFunctionType.Relu,
                                     bias=biasT_t[:, 4:5])
            pstage = stp.tile([P, nblk, 16], f32, tag="pstage")
            for b in range(nblk):
                ps = psp.tile([P, D], f32, tag="sm")
                nc.tensor.matmul(out=ps[:, :13], lhsT=p1T[:, b*P:(b+1)*P],
                                 rhs=wn2_t[:], start=True, stop=True)
                nc.vector.tensor_add(out=pstage[:, b, :13], in0=ps[:, :13],
                                     in1=bn2_tile[:])
            nc.sync.dma_start(
                out=pout[:].rearrange("(b p) d -> p b d", p=P),
                in_=pstage[:])

            # ---- h transpose to natural + output + gsum ----
            hstage = stp.tile([P, nblk, D], f32, tag="hstage")
            for b in range(nblk):
                ps = psp.tile([P, D], f32, tag="sm")
                nc.tensor.transpose(out=ps[:], in_=hT[:, b*P:(b+1)*P],
                                    identity=ident[:D, :D])
                nc.scalar.activation(out=hstage[:, b, :], in_=ps[:],
                                     func=mybir.ActivationFunctionType.Copy)
            nc.sync.dma_start(
                out=hout[:].rearrange("(b p) d -> p b d", p=P),
                in_=hstage[:])
            ps_g = pg.tile([D, 1], f32, tag="gps")
            for b in range(nblk):
                nc.tensor.matmul(out=ps_g[:], lhsT=hstage[:, b, :],
                                 rhs=mask_t[:, b:b+1],
                                 start=(b == 0), stop=(b == nblk - 1))
            gs = wp.tile([D, 1], f32, tag="gs")
            nc.scalar.activation(out=gs[:], in_=ps_g[:],
                                 func=mybir.ActivationFunctionType.Copy)
            nc.sync.dma_start(out=gout[:], in_=gs[:])

    nc.finalize()
    return nc


def _agg_update(nc, mybir, wp, hT, dinvT, biasT_t, ps_ag, blk, l, Npad):
    """h[:, blk] += relu(aggT * dinvT + bias_l)  (T layout, [64, 128] block)."""
    sl = slice(blk * P, (blk + 1) * P)
    t1 = wp.tile([64, P], mybir.dt.float32, tag="t1")
    nc.vector.tensor_mul(out=t1[:], in0=ps_ag[:], in1=dinvT[:, sl])
    t2 = wp.tile([64, P], mybir.dt.float32, tag="t2")
    nc.scalar.activation(out=t2[:], in_=t1[:],
                         func=mybir.ActivationFunctionType.Relu,
                         bias=biasT_t[:, l+1:l+2])
    nc.vector.tensor_add(out=hT[:, sl], in0=hT[:, sl], in1=t2[:])


# --------------------------------------------------------------------------
# kernel entry
# --------------------------------------------------------------------------
def kernel(x, edge_index, W_emb, b_emb, Wc1, bc1, Wc2, bc2, Wc3, bc3,
           Wn1, bn1, Wn2, bn2, Wg1, bg1, Wg2, bg2):
    from concourse.bass_utils import run_bass_kernel_spmd

    f32 = np.float32
    x = np.asarray(x, f32)
    edge_index = np.asarray(edge_index)
    ekey = hash(edge_index.tobytes())
    if _cache.get("ekey") == ekey:
        plan = _cache["plan"]
    else:
        plan = _build_plan(edge_index)
        _cache["ekey"] = ekey
        _cache["plan"] = plan
    Npad, n_chunks = plan["Npad"], plan["n_chunks"]
    nblk = Npad // P

    prog_key = (Npad, n_chunks, tuple(plan["pat_ids"].keys()),
                tuple(k for k, _, _ in plan["sched"]))
    if _cache.get("key") == prog_key:
        nc = _cache["nc"]
    else:
        nc = _build_program(plan)
        _cache["key"] = prog_key
        _cache["nc"] = nc

    pal_flat = np.ascontiguousarray(
        plan["pats"].transpose(1, 0, 2).reshape(P, -1)).astype(f32)
    biasT = np.zeros((D, 6), f32)
    for col, b in enumerate([b_emb, bc1, bc2, bc3, bn1]):
        biasT[:, col] = np.asarray(b, f32)
    bn2r = np.asarray(bn2, f32).reshape(1, 13)

    in_maps = []
    for c in range(C):
        pi = plan["per_core_pi"][c]
        xp = np.zeros((Npad, F), f32)
        real = pi >= 0
        xp[real] = x[pi[real]]
        dinv_pi = plan["dinv_pi"][c]
        in_maps.append({
            "xT": np.ascontiguousarray(xp.T),
            "idx": plan["idx"][c],
            "pal": pal_flat,
            "dinvnat": np.ascontiguousarray(
                dinv_pi.reshape(nblk, P).T).astype(f32),
            "dinvrow": dinv_pi.reshape(1, Npad),
            "biasT": biasT,
            "bn2r": bn2r,
            "maskc": np.ascontiguousarray(
                plan["mask"][c].reshape(nblk, P).T).astype(f32),
            "Wemb": np.asarray(W_emb, f32),
            "Wc1": np.asarray(Wc1, f32),
            "Wc2": np.asarray(Wc2, f32),
            "Wc3": np.asarray(Wc3, f32),
            "Wn1": np.asarray(Wn1, f32),
            "Wn2": np.asarray(Wn2, f32),
        })

    res = run_bass_kernel_spmd(nc, in_maps, list(range(C)))

    h_full = np.zeros((N, D), f32)
    preds = np.zeros((N, 13), f32)
    gsum = np.zeros(D, np.float64)
    for c in range(C):
        out = res.results[c]
        pi = plan["per_core_pi"][c]
        real = pi >= 0
        h_full[pi[real]] = out["h_out"][real]
        preds[pi[real]] = out["preds_out"][real][:, :13]
        gsum += out["gsum_out"][:, 0]

    g = (gsum / np.float64(N)).astype(f32)[None, :]
    Wg1 = np.asarray(Wg1, f32); bg1 = np.asarray(bg1, f32)
    Wg2 = np.asarray(Wg2, f32); bg2 = np.asarray(bg2, f32)
    z = np.maximum(g @ Wg1 + bg1, 0.0).astype(f32) @ Wg2 + bg2
    gp = (1.0 / (1.0 + np.exp(-z))).astype(f32)

    return (h_full, preds[:, :10], preds[:, 10:13], gp)
